# revision 1
# baseline (speedup 1.0000x reference)
"""Multi-head causal attention (RoPE) Trainium2 kernel v2, SPMD over 8 cores.

Same sharding as v1 (core = batch b x head-group g, 4 heads each, no
collectives), but the per-core schedule is a single interleaved PE stream:
attention for head h (scores -> exp on ACT -> PV) is emitted *between*
projection matmuls of later heads, so the ACT-engine exp work (the phase-B
bottleneck in v1) hides under projection matmuls instead of serializing
after them.  A virtual-clock pacing merger inserts projection "pad" units
between scores tiles and before PV groups so the PE never waits on exp.

Other changes vs v1:
 - startup: x/w tiles stream per-e interleaved across BOTH HWDGE rings
   (sync + scalar); consts + RoPE swap DMAs move to the gpsimd (SWDGE)
   ring; out DMAs alternate the HW rings.
 - QK head 0 is projected e-outer (Q/K interleaved per e-tile) so the
   first matmuls need only one x tile; 8 PSUM banks in a dedicated pool.
 - causal mask multiply narrowed to the 128-col diagonal block only.
 - PV of head 2 chunk 3 is deferred and used as PE pad work during the
   final head's attention (tail would otherwise be exp-bound).
"""

import sys

import numpy as np
import ml_dtypes

for _p in ("/opt/trn_rl_repo",):
    if _p not in sys.path:
        sys.path.insert(0, _p)

B, S, E = 2, 2048, 2048
H, D = 16, 128
P = 128
HPC = 4            # heads per core
F = HPC * D        # 512 projection features per core
NCORES = 8
NE = E // P        # 16 contraction tiles
NSQ = S // P       # 16 seq row-tiles
NCH = S // 512     # 4 chunks of 512
ROPE_BASE = 10000.0
SM_SCALE = 1.0 / float(np.sqrt(D))
BF16 = ml_dtypes.bfloat16

_compiled = None
LAST_RESULT = None

# interleaved (0,1),(2,3).. pairs -> half layout (i, i+64)
_PERM = np.concatenate([np.arange(0, D, 2), np.arange(1, D, 2)])

# virtual-clock cost estimates (ns)
MM512 = 260.0
MMPV = 80.0
IDENT = 830.0
SWAP_LAT = 2000.0
VEVAC = 900.0
PVEVAC = 650.0
MASK_DVE = 200.0
MARGIN = 400.0


def _mm_cost(w):
    return w / 2.4 + 46.0


def _exp_cost(w):
    return 180.0 + 1.667 * w


def _rope_tables():
    inv = ROPE_BASE ** (-np.arange(0, D, 2, dtype=np.float64) / D)
    ang = np.arange(S, dtype=np.float64)[None, :] * inv[:, None]
    cos, sin = np.cos(ang), np.sin(ang)
    cosf = np.concatenate([cos, cos], axis=0).astype(BF16)
    ssin = np.concatenate([-sin, sin], axis=0).astype(BF16)
    return cosf, ssin


def _mask_tile():
    # [128,128] lower-triangle-inclusive: mask[p, f] = 1 iff f >= p
    f = np.arange(P)[None, :]
    p = np.arange(P)[:, None]
    return (f >= p).astype(np.float32).astype(BF16)


def _build():
    import concourse.mybir as mybir
    import concourse.tile as tile
    from concourse import bacc

    fdt = mybir.dt.float32
    bdt = mybir.dt.bfloat16
    Exp = mybir.ActivationFunctionType.Exp
    Ident = mybir.ActivationFunctionType.Identity

    nc = bacc.Bacc("TRN2", target_bir_lowering=False, debug=False,
                   num_devices=NCORES)

    xt = nc.dram_tensor("xt", [E, S], bdt, kind="ExternalInput").ap()
    wqt = nc.dram_tensor("wqt", [E, F], bdt, kind="ExternalInput").ap()
    wkt = nc.dram_tensor("wkt", [E, F], bdt, kind="ExternalInput").ap()
    wvt = nc.dram_tensor("wvt", [E, F], bdt, kind="ExternalInput").ap()
    bqd = nc.dram_tensor("bqd", [P, HPC], fdt, kind="ExternalInput").ap()
    bkd = nc.dram_tensor("bkd", [P, HPC], fdt, kind="ExternalInput").ap()
    bvbd = nc.dram_tensor("bvbd", [P, F], fdt, kind="ExternalInput").ap()
    cosd = nc.dram_tensor("cosd", [P, S], bdt, kind="ExternalInput").ap()
    ssind = nc.dram_tensor("ssind", [P, S], bdt, kind="ExternalInput").ap()
    maskd = nc.dram_tensor("maskd", [P, P], bdt, kind="ExternalInput").ap()
    outd = nc.dram_tensor("out", [S, F], fdt, kind="ExternalOutput").ap()

    with tile.TileContext(nc) as tc:
        with (
            tc.tile_pool(name="const", bufs=1) as constp,
            tc.tile_pool(name="xp", bufs=1) as xp,
            tc.tile_pool(name="wvp", bufs=1) as wvp,
            tc.tile_pool(name="wqk", bufs=2) as wqk,
            tc.tile_pool(name="qk", bufs=2) as qkp,
            tc.tile_pool(name="va", bufs=1) as vap,
            tc.tile_pool(name="evac", bufs=6) as ep,
            tc.tile_pool(name="w0p", bufs=1) as w0p,
            tc.tile_pool(name="et2", bufs=2) as etp,
            tc.tile_pool(name="et3", bufs=1) as et3p,
            tc.tile_pool(name="etd", bufs=1) as etdp,
            tc.tile_pool(name="etd2", bufs=1) as etdp2,
            tc.tile_pool(name="ost", bufs=6) as osp,
        ):
            # HAM warmup source: the PE runs dependency-free dummy matmuls
            # during the startup DMA wait so the clock gate is at 8/8 when
            # the first real matmul issues (saves ~6us of half-clock MMs)
            dum = constp.tile([P, 512], bdt, tag="dum", name="dum")
            nc.vector.memset(dum[:], 0.0)

            # ---- small constants first on the gpsimd (SWDGE) ring ----
            bqpt = constp.tile([P, HPC], fdt, tag="bqpt", name="bqpt")
            nc.gpsimd.dma_start(bqpt[:], bqd[:])
            bkpt = constp.tile([P, HPC], fdt, tag="bkpt", name="bkpt")
            nc.gpsimd.dma_start(bkpt[:], bkd[:])
            mask_sb = constp.tile([P, P], bdt, tag="mask", name="mask_sb")
            nc.gpsimd.dma_start(mask_sb[:], maskd[:])

            # ---- startup streaming ----
            # sync ring: batched head-0 weights, x-even, batched wv, lazy
            # later-head weights, head-0 swaps, head-0..2 out tiles.
            # scalar ring (ACT idle early): x-odd first, head-3 outs late.
            # gpsimd SWDGE: consts, later-head swaps.
            def wload(dram, h, nm):
                # one DMA for all 16 [128,128] e-tiles of head h: SBUF
                # [128, 2048] where cols e*128+f <- dram[128e+p, 128h+f]
                t = wqk.tile([P, S], bdt, tag=f"w{nm[0]}", name=nm)
                src = dram[:, P * h:P * (h + 1)].rearrange(
                    "(e p) f -> p e f", p=P)
                nc.sync.dma_start(t[:].rearrange("p (e f) -> p e f", e=NE),
                                  src)
                return t

            # head-0 weights as small per-e tiles interleaved with x-even on
            # sync so the first matmul can start ~10us in; x-odd + wv halves
            # on the scalar ring (ACT is idle until the first evac anyway).
            xts = [None] * NE
            wv_all = wvp.tile([P, NE * F], bdt, tag="wv", name="wv_all")
            # x-odd on the SWDGE ring, wv alone on the scalar ring (it
            # drains in ~12us, so the ACT engine is free well before the
            # first PSUM evacuation needs it)
            for e in range(NE):
                if e % 2 == 1:
                    t = xp.tile([P, S], bdt, tag=f"x{e}", name=f"x{e}")
                    nc.gpsimd.dma_start(t[:], xt[P * e:P * (e + 1), :])
                    xts[e] = t
            # RoPE tables + v bias after x-odd (not needed until ~45us)
            cos_sb = constp.tile([P, S], bdt, tag="cos", name="cos_sb")
            nc.gpsimd.dma_start(cos_sb[:], cosd[:])
            ssin_sb = constp.tile([P, S], bdt, tag="ssin", name="ssin_sb")
            nc.gpsimd.dma_start(ssin_sb[:], ssind[:])
            bvb_sb = constp.tile([P, F], fdt, tag="bvb", name="bvb_sb")
            nc.gpsimd.dma_start(bvb_sb[:], bvbd[:])
            # head-0 weight tiles split across both HW rings so the sync
            # ring's issue-serialization doesn't delay the x-even stream
            # (the scalar ring is otherwise idle until wv below)
            w0q = [None] * NE
            w0k = [None] * NE
            for e in range(NE):
                r = nc.sync if e % 2 == 0 else nc.scalar
                t = w0p.tile([P, P], bdt, tag=f"q{e}", name=f"w0q{e}")
                r.dma_start(t[:], wqt[P * e:P * (e + 1), 0:P])
                w0q[e] = t
                t = w0p.tile([P, P], bdt, tag=f"k{e}", name=f"w0k{e}")
                r.dma_start(t[:], wkt[P * e:P * (e + 1), 0:P])
                w0k[e] = t
                if e % 2 == 0:
                    t = xp.tile([P, S], bdt, tag=f"x{e}", name=f"x{e}")
                    nc.sync.dma_start(t[:], xt[P * e:P * (e + 1), :])
                    xts[e] = t
            # wv after the w0 tiles on the scalar ring (needed only ~45us)
            nc.scalar.dma_start(
                wv_all[:, 0:8 * F].rearrange("p (e f) -> p e f", e=8),
                wvt[0:8 * P, :].rearrange("(e p) f -> p e f", p=P))
            nc.scalar.dma_start(
                wv_all[:, 8 * F:].rearrange("p (e f) -> p e f", e=8),
                wvt[8 * P:, :].rearrange("(e p) f -> p e f", p=P))

            wq_all = {}
            wk_all = {}

            def load_wqk(h):
                wq_all[h] = wload(wqt, h, f"wq{h}")
                wk_all[h] = wload(wkt, h, f"wk{h}")

            qT = {}
            kT = {}
            vA = [[vap.tile([P, D + 1], bdt, tag=f"vA{h}_{j}",
                            name=f"vA{h}_{j}")
                   for j in range(NSQ)] for h in range(HPC)]
            for h in range(HPC):
                for j in range(NSQ):
                    nc.vector.memset(vA[h][j][:, D:D + 1], 1.0)

            # virtual clocks (ns)
            clk = {"pe": 0.0, "act": 0.0, "dve": 0.0}

            def on_pe(cost):
                clk["pe"] += cost

            def on_act(cost):
                clk["act"] = max(clk["act"], clk["pe"]) + cost

            def on_dve(cost, ready=None):
                base = max(clk["dve"], clk["pe"] if ready is None else ready)
                clk["dve"] = base + cost

            # ---- evac + RoPE chain for a finished QK psum bank ----
            # head-0 swaps go on the fast sync HWDGE ring (latency-critical:
            # scores(0,*) wait on them); later heads' swaps are prefetched
            # well ahead, so the slow SWDGE ring is fine and keeps sync free.
            def qk_evac(ps_t, h, c, is_q):
                cs = slice(512 * c, 512 * (c + 1))
                dst = qT[h] if is_q else kT[h]
                bias = bqpt if is_q else bkpt
                nm = f"{'q' if is_q else 'k'}{h}_{c}"
                xs = ep.tile([P, 512], bdt, tag="xs", name=f"xs{nm}")
                nc.scalar.activation(xs[:], ps_t[:], Ident,
                                     bias=bias[:, h:h + 1])
                on_act(IDENT)
                sw = ep.tile([P, 512], bdt, tag="sw", name=f"sw{nm}")
                swr = nc.sync if h == 0 else nc.gpsimd
                swr.dma_start(sw[0:64, :], xs[64:128, :])
                swr.dma_start(sw[64:128, :], xs[0:64, :])
                t2 = ep.tile([P, 512], bdt, tag="t2", name=f"t2{nm}")
                nc.vector.tensor_mul(dst[:, cs], xs[:], cos_sb[:, cs])
                nc.vector.tensor_mul(t2[:], sw[:], ssin_sb[:, cs])
                nc.vector.tensor_add(dst[:, cs], dst[:, cs], t2[:])
                on_dve(3 * 500.0, ready=clk["act"] + SWAP_LAT)

            # ================= QK head 0: e-outer, 6 banks =================
            # Q c0-3 + K c0-1 accumulate e-outer (only needs one x tile per
            # step, so the DMA stream feeds it); K c2/c3 are done later as
            # ordinary pad units once x is resident.  Evacs stagger so
            # scores(0,0) can start ~25us earlier than an 8-bank block.
            with tc.tile_pool(name="ps0", bufs=1, space="PSUM") as ps0:
                qT[0] = qkp.tile([P, S], bdt, tag="qT", name="qT0")
                kT[0] = qkp.tile([P, S], bdt, tag="kT", name="kT0")
                bank = {}
                parts = [(m, c) for c in range(NCH) for m in ("q", "k")]
                for m, c in parts:
                    bank[(m, c)] = ps0.tile([P, 512], fdt, tag=f"b{m}{c}",
                                            name=f"ps{m}0_{c}")
                for i in range(48):
                    nc.tensor.matmul(bank[("q", 0)][:], dum[:, 0:P],
                                     dum[:], start=True, stop=True)
                for e in range(NE - 4):
                    for m, c in parts:
                        wt = w0q if m == "q" else w0k
                        nc.tensor.matmul(
                            bank[(m, c)][:], wt[e][:],
                            xts[e][:, 512 * c:512 * (c + 1)],
                            start=(e == 0), stop=False)
                        on_pe(MM512)
                # staggered finals: finish one (m, c) bank at a time and
                # evacuate it immediately, so RoPE/scores(0,0) start ~10us
                # earlier than a single joint finish would allow
                for m, c in parts:
                    wt = w0q if m == "q" else w0k
                    for e in range(NE - 4, NE):
                        nc.tensor.matmul(
                            bank[(m, c)][:], wt[e][:],
                            xts[e][:, 512 * c:512 * (c + 1)],
                            start=False, stop=(e == NE - 1))
                        on_pe(MM512)
                    qk_evac(bank[(m, c)], 0, c, m == "q")

            # ================= main pools + pad-unit machinery ============
            with tc.tile_pool(name="psm", bufs=1, space="PSUM") as psm:
                # alloc order maps tiles to the physical banks freed earliest
                # by the interleaved q/k evac order above
                vvb = [psm.tile([P, 512], fdt, tag=f"vv{i}", name=f"vv{i}")
                       for i in range(2)]
                scb = [psm.tile([P, 512], fdt, tag=f"sc{i}", name=f"sc{i}")
                       for i in range(2)]
                prj = [psm.tile([P, 512], fdt, tag=f"prj{i}", name=f"prj{i}")
                       for i in range(2)]
                pob = [psm.tile([P, D + 1], fdt, tag=f"po{i}", name=f"po{i}")
                       for i in range(2)]

                units = []          # (key, cost, fn)
                emitted = set()

                def emit_one():
                    key, cost, fn = units.pop(0)
                    fn()
                    on_pe(cost)
                    emitted.add(key)

                def pad_until(target):
                    while clk["pe"] < target and units:
                        emit_one()

                def flush_until(key):
                    while key not in emitted and units:
                        emit_one()

                # --- unit builders ---
                def v_unit(j, e0):
                    def fn(j=j, e0=e0):
                        vb = vvb[j % 2]
                        for e in range(e0, e0 + 4):
                            nc.tensor.matmul(
                                vb[:], xts[e][:, P * j:P * (j + 1)],
                                wv_all[:, F * e:F * (e + 1)],
                                start=(e == 0), stop=(e == NE - 1))
                        if e0 == 12:
                            for h in range(HPC):
                                nc.vector.tensor_copy(
                                    vA[h][j][:, 0:D], vb[:, D * h:D * (h + 1)])
                            on_dve(VEVAC)
                    return (("v", j, e0), 4 * MM512, fn)

                def qk_unit(m, h, c, e0, mk=False):
                    def fn(m=m, h=h, c=c, e0=e0, mk=mk):
                        if mk and m == "q":
                            qT[h] = qkp.tile([P, S], bdt, tag="qT",
                                             name=f"qT{h}")
                        if mk and m == "k":
                            kT[h] = qkp.tile([P, S], bdt, tag="kT",
                                             name=f"kT{h}")
                        pt = prj[0] if m == "q" else prj[1]
                        for e in range(e0, e0 + 4):
                            if h == 0:
                                lhs = (w0q if m == "q" else w0k)[e][:]
                            else:
                                wt = wq_all[h] if m == "q" else wk_all[h]
                                lhs = wt[:, P * e:P * (e + 1)]
                            nc.tensor.matmul(
                                pt[:], lhs,
                                xts[e][:, 512 * c:512 * (c + 1)],
                                start=(e == 0), stop=(e == NE - 1))
                        if e0 == 12:
                            qk_evac(pt, h, c, m == "q")
                    return ((m, h, c, e0), 4 * MM512, fn)

                def queue_v(j):
                    for e0 in (0, 4, 8, 12):
                        units.append(v_unit(j, e0))

                def queue_qk(h):
                    load_wqk(h)
                    if h == 3:
                        # K first (scores(3,c) need all K chunks <= c), Q in
                        # descending-c order matching head-3's chunk order:
                        # leftover Q units double as tail PE pad work.
                        seq = [("k", c) for c in range(NCH)]
                        seq += [("q", c) for c in (3, 2, 1, 0)]
                    else:
                        seq = [(m, c) for c in range(NCH)
                               for m in ("q", "k")]
                    seen = set()
                    for m, c in seq:
                        for e0 in (0, 4, 8, 12):
                            units.append(
                                qk_unit(m, h, c, e0,
                                        mk=(m not in seen and e0 == 0)))
                        seen.add(m)

                # --- attention ---
                def attn(h, c, defer_pv=False, pool=None):
                    pool = pool or etp
                    nt = 4 * c + 4
                    if h == 3:
                        flush_until(("q", h, c, 12))
                    elif h > 0:
                        flush_until(("k", h, c, 12))
                    order = list(range(4 * c, nt)) + list(range(0, 4 * c))
                    sel = [None] * nt     # (tile, col_offset)
                    selm = [None] * nt    # masked diag block [P,128]
                    for t in order:
                        diag = t >= 4 * c
                        o = P * (t % 4) if diag else 0
                        w = 512 - o
                        cs = slice(512 * c + o, 512 * (c + 1))
                        ps_sc = scb[t % 2]
                        nc.tensor.matmul(ps_sc[:, 0:w],
                                         kT[h][:, P * t:P * (t + 1)],
                                         qT[h][:, cs], start=True, stop=True)
                        on_pe(_mm_cost(w))
                        tp = pool if pool is etdp else (
                            et3p if t >= 12 else etp)
                        et = tp.tile([P, w], bdt, tag=f"et{t}",
                                     name=f"et{h}_{c}_{t}")
                        nc.scalar.activation(et[:], ps_sc[:, 0:w], Exp,
                                             scale=SM_SCALE)
                        on_act(_exp_cost(w))
                        sel[t] = (et, o)
                        if diag:
                            etm = tp.tile([P, P], bdt, tag=f"etm{t % 4}",
                                          name=f"etm{h}_{c}_{t}")
                            nc.vector.tensor_mul(etm[:], et[:, 0:P],
                                                 mask_sb[:])
                            on_dve(MASK_DVE)
                            selm[t] = etm
                        pad_until(clk["act"] - 2 * _exp_cost(512))
                    gate = max(clk["act"], clk["dve"])
                    if defer_pv:
                        return sel, selm
                    if h == 0:
                        flush_until(("v", 4 * c + 3, 12))
                    pad_until(gate + MARGIN)
                    pv(h, c, sel, selm)

                def pv(h, c, sel, selm, as_units=False):
                    mk = []
                    for jj in range(4):
                        def fn(jj=jj, h=h, c=c, sel=sel, selm=selm):
                            j = 4 * c + jj
                            po = pob[jj % 2]
                            for t in range(j + 1):
                                if t == j:
                                    src = selm[t][:]
                                else:
                                    et, o = sel[t]
                                    lo = P * jj - o
                                    src = et[:, lo:lo + P]
                                nc.tensor.matmul(po[:], src, vA[h][t][:],
                                                 start=(t == 0), stop=(t == j))
                            rec = osp.tile([P, 1], fdt, tag="rec",
                                           name=f"rec{h}_{j}")
                            nc.vector.reciprocal(rec[:], po[:, D:D + 1])
                            ot = osp.tile([P, D], fdt, tag="ot",
                                          name=f"ot{h}_{j}")
                            nc.vector.scalar_tensor_tensor(
                                ot[:], po[:, 0:D], rec[:],
                                bvb_sb[:, D * h:D * (h + 1)],
                                mybir.AluOpType.mult, mybir.AluOpType.add)
                            on_dve(PVEVAC)
                            r = nc.scalar if (h == 3 or as_units) else nc.sync
                            r.dma_start(
                                outd[P * j:P * (j + 1), D * h:D * (h + 1)],
                                ot[:])
                        cost = (4 * c + jj + 1) * MMPV
                        if as_units:
                            mk.append((("pvd", h, c, jj), cost, fn))
                        else:
                            fn()
                            on_pe(cost)
                    return mk

                # ---------------- the schedule ----------------
                for j in range(NSQ):
                    queue_v(j)
                queue_qk(1)
                # bridge the head-0 RoPE-chain latency with V work so the
                # PE isn't head-of-line blocked on scores(0,0)'s inputs
                flush_until(("v", 1, 12))
                for c in range(NCH):
                    attn(0, c)
                queue_qk(2)
                d1 = None
                for c in range(NCH):
                    if c == 3:
                        d1 = attn(1, c, defer_pv=True, pool=etdp2)
                    else:
                        attn(1, c)
                queue_qk(3)
                dsel = dselm = None
                for c in range(NCH):
                    if c == 3:
                        dsel, dselm = attn(2, c, defer_pv=True, pool=etdp)
                    else:
                        attn(2, c)
                # head 3 descending, deferred PV(1,3)+PV(2,3) as tail pads
                units.extend(pv(1, 3, d1[0], d1[1], as_units=True))
                units.extend(pv(2, 3, dsel, dselm, as_units=True))
                g3 = {}
                for c in (3, 2, 1, 0):
                    s, m = attn(3, c, defer_pv=True)
                    g3[c] = (s, m, max(clk["act"], clk["dve"]))
                    if c == 3:
                        continue
                    sd, md, gate = g3[c + 1]
                    if c == 0:
                        # spend reserve pads now: the kernel must end on
                        # dependency-free matmuls, not an exp wait
                        while len(units) > 2:
                            emit_one()
                    pad_until(gate + MARGIN)
                    pv(3, c + 1, sd, md)
                sd, md, gate = g3[0]
                while units:
                    emit_one()
                pv(3, 0, sd, md)

    nc.compile()
    return nc


def get_compiled():
    global _compiled
    if _compiled is None:
        _compiled = _build()
    return _compiled


def expected_slice_core0(expected):
    return expected[0, :, 0:F]


def make_in_maps(logits, Wq, bq, Wk, bk, Wv, bv):
    cosf, ssin = _rope_tables()
    maskm = _mask_tile()
    xts = [np.ascontiguousarray(np.asarray(logits)[b].T).astype(BF16)
           for b in range(B)]

    def permW(Wm, rows):
        Wp = np.asarray(Wm)[rows].reshape(HPC, D, E)[:, _PERM, :].reshape(F, E)
        return np.ascontiguousarray(Wp.T).astype(BF16)

    def permb(bvec, rows):
        return np.ascontiguousarray(
            np.asarray(bvec)[rows].reshape(HPC, D)[:, _PERM].T
        ).astype(np.float32)

    in_maps = []
    for core in range(NCORES):
        b, g = divmod(core, 4)
        rows = slice(F * g, F * (g + 1))
        in_maps.append({
            "xt": xts[b],
            "wqt": permW(Wq, rows),
            "wkt": permW(Wk, rows),
            "wvt": np.ascontiguousarray(np.asarray(Wv)[rows].T).astype(BF16),
            "bqd": permb(bq, rows),
            "bkd": permb(bk, rows),
            "bvbd": np.ascontiguousarray(np.broadcast_to(
                np.asarray(bv)[rows].astype(np.float32), (P, F))),
            "cosd": cosf,
            "ssind": ssin,
            "maskd": maskm,
        })
    return in_maps


def kernel(logits, Wq, bq, Wk, bk, Wv, bv, **_ignored):
    global LAST_RESULT
    from concourse.bass_utils import run_bass_kernel_spmd

    nc = get_compiled()
    in_maps = make_in_maps(logits, Wq, bq, Wk, bk, Wv, bv)
    res = run_bass_kernel_spmd(nc, in_maps, list(range(NCORES)))
    LAST_RESULT = res
    out = np.empty((B, S, H * D), dtype=np.float32)
    for core in range(NCORES):
        b, g = divmod(core, 4)
        out[b, :, F * g:F * (g + 1)] = res.results[core]["out"]
    return out

